# revision 14
# baseline (speedup 1.0000x reference)
"""Trainium2 Bass kernel for nn_AGCB_Element (sparse_attention).

Sharding: pure data parallel over (batch=2) x (2x2 spatial blocks) = 8
cores; one (batch, block) non-local attention unit per core, fully
SBUF/PSUM-resident. Params replicated. Two tiny AllGathers per batch
group of 4 cores: pooled 2x2 maxima (for the GCA branch, computed
redundantly per group) and gated-context halo edges (for the 3x3 conv).

SPMD uniformity: all cores run one graph, so per-core spatial geometry
is normalized by flipping x/y of the inputs on the host (conv weights,
upsample matrix, x tile flipped as data; outputs unflipped). Halo
neighbor selection uses per-core 0/1 mask input tensors.

Attention per core (N=4096, inter=2), transposed-layout softmax:
  Lt[m,n] = k^T q;  Et = exp(Lt)  (no max subtraction: |Lt| < ~14);
  out' = [v; 1]^T Et  -> row 64 is the denominator Z[n];
  ctx = num * (sig * nl_gamma / Z) + sig * x.

Raw bass (explicit engines/semaphores) - the Tile framework emits
multi-wait instructions this walrus build rejects.
"""
import sys

if "/opt/trn_rl_repo" not in sys.path:
    sys.path.insert(0, "/opt/trn_rl_repo")

from contextlib import ExitStack

import numpy as np

import concourse.bass as bass
import concourse.mybir as mybir
from concourse.bass_utils import run_bass_kernel_spmd

C = 64
HB = WB = 64
N = HB * WB            # 4096 spatial positions per block
NCH = 4                # n-chunks
CW = N // NCH          # 1024
MT = 32                # m-tiles of 128
EPS = 1e-5
F32 = mybir.dt.float32
AF = mybir.ActivationFunctionType
ALU = mybir.AluOpType
AX = mybir.AxisListType
GROUPS4 = [[0, 1, 2, 3], [4, 5, 6, 7]]


def _interp_w(n_out, n_in=2):
    ys = np.linspace(0.0, n_in - 1.0, n_out)
    y0 = np.clip(np.floor(ys).astype(np.int64), 0, n_in - 1)
    y1 = np.minimum(y0 + 1, n_in - 1)
    wy = ys - y0
    W = np.zeros((n_out, n_in), np.float64)
    for r in range(n_out):
        W[r, y0[r]] += 1.0 - wy[r]
        W[r, y1[r]] += wy[r]
    return W.astype(np.float32)


def prep_inputs(inputs):
    """Host-side sharding + parameter prep. Returns (in_maps, scalars)."""
    f32 = np.float32
    x = np.asarray(inputs['x'])
    c65 = np.zeros((C + 1, 142), f32)
    c65[0:4, 138:142] = np.eye(4, dtype=f32)
    c65[:, 0:2] = np.concatenate([np.asarray(inputs['nl_q_w']).T,
                                  np.asarray(inputs['nl_q_b'])[None, :]], 0)
    c65[:, 2:4] = np.concatenate([np.asarray(inputs['nl_k_w']).T,
                                  np.asarray(inputs['nl_k_b'])[None, :]], 0)
    c65[:, 4:6] = np.concatenate([np.asarray(inputs['gca_q_w']).T,
                                  np.asarray(inputs['gca_q_b'])[None, :]], 0)
    c65[:, 6:8] = np.concatenate([np.asarray(inputs['gca_k_w']).T,
                                  np.asarray(inputs['gca_k_b'])[None, :]], 0)
    rhs65 = np.zeros((C + 1, C + 1), f32)
    rhs65[:C, :C] = np.asarray(inputs['nl_v_w']).T
    rhs65[C, :C] = np.asarray(inputs['nl_v_b'])
    rhs65[C, C] = 1.0
    c65[:, 8:73] = rhs65
    grhs65 = np.zeros((C + 1, C + 1), f32)
    grhs65[:C, :C] = np.asarray(inputs['gca_v_w']).T
    grhs65[C, :C] = np.asarray(inputs['gca_v_b'])
    grhs65[C, C] = 1.0
    c65[:, 73:138] = grhs65

    nl_gamma = float(inputs['nl_gamma'])
    gca_gamma = float(inputs['gca_gamma'])
    gamma = float(inputs['gamma'])
    scale = np.asarray(inputs['bn_w']) / np.sqrt(np.asarray(inputs['bn_var']) + EPS)
    Wc = np.asarray(inputs['conv_w']) * scale[:, None, None, None]
    bc = ((np.asarray(inputs['conv_b']) - np.asarray(inputs['bn_mean'])) * scale
          + np.asarray(inputs['bn_b']))
    b2 = (gamma * bc).astype(f32).reshape(C, 1)
    grow = np.full((1, C), nl_gamma, f32)
    Wy = _interp_w(2 * HB)
    Wx = _interp_w(2 * WB)

    in_maps = []
    for core in range(8):
        b, blk = core // 4, core % 4
        i0, j0 = blk // 2, blk % 2
        fy, fx = (i0 == 1), (j0 == 1)
        xt = x[b, :, i0 * HB:(i0 + 1) * HB, j0 * WB:(j0 + 1) * WB]
        if fy:
            xt = xt[:, ::-1, :]
        if fx:
            xt = xt[:, :, ::-1]
        xt = np.ascontiguousarray(xt).reshape(C, N).astype(f32)
        Wcf = Wc
        if fy:
            Wcf = Wcf[:, :, ::-1, :]
        if fx:
            Wcf = Wcf[:, :, :, ::-1]
        wconv = np.ascontiguousarray(Wcf.transpose(1, 2, 3, 0)).reshape(C, 9 * C).astype(f32)
        Wy_t = Wy[i0 * HB:(i0 + 1) * HB]
        Wx_t = Wx[j0 * WB:(j0 + 1) * WB]
        if fy:
            Wy_t = Wy_t[::-1]
        if fx:
            Wx_t = Wx_t[::-1]
        m_up = np.einsum('pi,qj->ijpq', Wy_t, Wx_t).reshape(4, N).astype(f32)
        r_h, r_v, r_d = blk ^ 1, blk ^ 2, blk ^ 3
        hmask = np.zeros((C, 4, 129), f32)
        hmask[:, r_h, 0:WB] = 1.0
        hmask[:, r_v, WB:2 * WB] = 1.0
        hmask[:, r_d, 2 * WB] = 1.0
        in_maps.append(dict(
            x_tile=xt, c65=c65, grow=grow, b2=b2, m_up=m_up, wconv=wconv,
            hmask=np.ascontiguousarray(hmask.reshape(C, 4 * 129))))
    return in_maps, dict(nl_gamma=nl_gamma, gca_gamma=gca_gamma, gamma=gamma)


def unshard(outs):
    f32 = np.float32
    out = np.zeros((2, C, 2 * HB, 2 * WB), f32)
    for core in range(8):
        b, blk = core // 4, core % 4
        i0, j0 = blk // 2, blk % 2
        t = np.asarray(outs[core]).reshape(C, HB, WB)
        if i0 == 1:
            t = t[:, ::-1, :]
        if j0 == 1:
            t = t[:, :, ::-1]
        out[b, :, i0 * HB:(i0 + 1) * HB, j0 * WB:(j0 + 1) * WB] = t
    return out


def build_nc(nl_gamma, gca_gamma, gamma):
    nc = bass.Bass(num_devices=8)
    ctx = ExitStack()

    x_ext = nc.declare_dram_parameter("x_tile", [C, N], F32, isOutput=False)
    c65_ext = nc.declare_dram_parameter("c65", [C + 1, 142], F32, isOutput=False)
    grow_ext = nc.declare_dram_parameter("grow", [1, C], F32, isOutput=False)
    b2_ext = nc.declare_dram_parameter("b2", [C, 1], F32, isOutput=False)
    mup_ext = nc.declare_dram_parameter("m_up", [4, N], F32, isOutput=False)
    wconv_ext = nc.declare_dram_parameter("wconv", [C, 9 * C], F32, isOutput=False)
    hmask_ext = nc.declare_dram_parameter("hmask", [C, 4 * 129], F32, isOutput=False)
    out_ext = nc.declare_dram_parameter("out", [C, N], F32, isOutput=True)

    pool_send = nc.dram_tensor("pool_send", [C], F32)
    pool_gath = nc.dram_tensor("pool_gath", [4, C], F32)
    halo_send = nc.dram_tensor("halo_send", [C, 129], F32)
    halo_gath = nc.dram_tensor("halo_gath", [4 * C, 129], F32)

    # ---- u-bank production schedule (PE order) ----
    prods = []
    prods += [("q", c) for c in range(8)]
    prods += [("k", c) for c in range(8)]
    prods += [("vt", t) for t in range(MT)]
    prods += [("gat",), ("vgt",), ("gq",), ("gk",), ("ltg",), ("outg",)]
    prods += [("up", c) for c in range(8)]
    for nci in range(NCH):
        prods += [("rb", nci, 0), ("rb", nci, 1)]
    prods += [("conv", c) for c in range(8)]
    P = {p: i for i, p in enumerate(prods)}
    NCONV0 = P[("conv", 0)]

    _names = [0]

    def sb(shape, name=None):
        _names[0] += 1
        return ctx.enter_context(nc.sbuf_tensor(name or f"sb{_names[0]}", shape, F32))

    def ps(shape):
        _names[0] += 1
        return ctx.enter_context(nc.psum_tensor(f"ps{_names[0]}", shape, F32))

    sem = lambda name: ctx.enter_context(nc.semaphore(name))

    xba = sb([C + 1, N])
    q_sb = sb([2, N])
    k_sb = sb([2, N])
    vt_sb = sb([128, MT * 65])
    et = [sb([128, CW]), sb([128, CW])]
    c65_sb = sb([C + 1, 142])
    grow_sb = sb([1, C])
    b2_sb = sb([C, 1])
    mup_sb = sb([4, N])
    wconv_sb = sb([C, 9 * C])
    hmask_sb = sb([C, 4 * 129])
    pooled_sb = sb([C, 1])
    gaug_sb = sb([C + 1, 4])
    gt_sb = sb([4, C])
    qg_sb = sb([2, 4])
    kg_sb = sb([2, 4])
    etg_sb = sb([4, 4])
    vgt_sb = sb([4, 65])
    numt_sb = sb([4, C])
    rg_sb = sb([4, 1])
    gtmp_sb = sb([4, C])
    gpt_sb = sb([4, C])
    e_sb = sb([C, N])
    sig_sb = sb([C, N])
    scr_sb = sb([C, N])
    p1_sb = sb([C, N])
    r_sb = sb([1, CW])
    zg_sb = sb([4, 1])
    s2_sb = sb([C, CW])
    g_sb = sb([C, CW])
    xc = sb([C, HB + 2, WB + 2])
    hg_sb = sb([C, 4, 129])
    hsend_sb = sb([C, 129])
    hA = sb([C, 129])
    hB = sb([C, 129])
    hC = sb([C, 129])
    hD = sb([C, 129])
    tA = [sb([C, 512]), sb([C, 512])]
    t2 = [sb([C, 512]), sb([C, 512])]
    osb = [sb([C, 512]), sb([C, 512])]

    lt = [ps([128, CW]), ps([128, CW])]   # psum banks 0-1, 2-3
    po = ps([128, CW])                    # banks 4-5
    u = [ps([128, 512]), ps([128, 512])]  # banks 6, 7

    sXIN = sem("sXIN")      # x_tile DMA (16)
    sIN = sem("sIN")        # const input DMAs (6 x 16 = 96)
    sMS = sem("sMS")        # DVE memsets: xba ones (1), gaug ones (2)
    sPOOL = sem("sPOOL")
    sPSEND = sem("sPSEND")
    sCC = sem("sCC")        # collectives: pool AG (1), halo AG (2)
    sGIN = sem("sGIN")      # gaug+gt DMAs (32)
    sL = sem("sL")          # PE L-pair per mt
    sE = sem("sE")          # ACT exp per mt
    sOA = sem("sOA")        # PE out'-pair per mt
    sR = sem("sR")          # DVE r ready (per nci)
    sEPo = sem("sEPo")      # DVE done reading po (per nci)
    sGATED = sem("sGATED")  # DVE gated chunk written (per nci)
    sUP = sem("sUP")        # PE u-bank production counter
    uc = [sem("u0c"), sem("u1c")]  # per-u-bank consumption counters
    sGCAdve = sem("sGCAdve")
    sZG = sem("sZG")
    sRG = sem("sRG")
    sSD = sem("sSD")
    sHS = sem("sHS")        # DVE packed halo-send strip (1)
    sHSEND = sem("sHSEND")  # halo-send DMA (16)
    sHG = sem("sHG")        # halo_gath -> sbuf DMA (16)
    sHALO = sem("sHALO")    # DVE halo merged into xc (1)
    sT2 = sem("sT2")        # DVE conv epilogue t2 per chunk
    sOUT = sem("sOUT")      # ACT relu per chunk
    sOD = [sem("sOD0"), sem("sOD1")]  # out DMAs by chunk parity

    def u_wait(eng, p):
        if p >= 2:
            eng.wait_ge(uc[p % 2], p // 2)

    def cons_through(p):
        # (bank0, bank1) consumption counts once productions 0..p are all
        # consumed
        n0 = sum(1 for i in range(p + 1) if i % 2 == 0)
        return n0, (p + 1) - n0

    def wait_consumed(eng, p):
        n0, n1 = cons_through(p)
        eng.wait_ge(uc[0], n0)
        eng.wait_ge(uc[1], n1)

    # ---------------- engine programs ----------------
    with nc.Block() as block:

        @block.sync
        def _(sy):
            sy.dma_start(out=xba[0:C, :], in_=x_ext[:]).then_inc(sXIN, 16)
            sy.dma_start(out=c65_sb[:], in_=c65_ext[:]).then_inc(sIN, 16)
            sy.dma_start(out=grow_sb[:], in_=grow_ext[:]).then_inc(sIN, 16)
            sy.dma_start(out=b2_sb[:], in_=b2_ext[:]).then_inc(sIN, 16)
            sy.dma_start(out=mup_sb[:], in_=mup_ext[:]).then_inc(sIN, 16)
            sy.dma_start(out=wconv_sb[:], in_=wconv_ext[:]).then_inc(sIN, 16)
            sy.dma_start(out=hmask_sb[:], in_=hmask_ext[:]).then_inc(sIN, 16)
            # pooled maxima out
            sy.wait_ge(sPOOL, 1)
            sy.dma_start(out=pool_send[:], in_=pooled_sb[:, 0:1]).then_inc(sPSEND, 16)
            # gathered pool back in (transposed into [c, n] + direct [n, c])
            sy.wait_ge(sCC, 1)
            sy.dma_start(out=gt_sb[:], in_=pool_gath[:]).then_inc(sGIN, 16)
            # halo send (uniform: local col 63, row 63, corner),
            # packed contiguously by DVE first
            sy.wait_ge(sHS, 1)
            sy.dma_start(out=halo_send[:], in_=hsend_sb[:]).then_inc(sHSEND, 16)
            # gathered halo in
            sy.wait_ge(sCC, 2)
            sy.dma_start(out=hg_sb[:],
                         in_=halo_gath[:].rearrange("(s c) j -> c s j", s=4)).then_inc(sHG, 16)
            # outputs
            for cch in range(8):
                sy.wait_ge(sOUT, cch + 1)
                sy.dma_start(out=out_ext[:, 512 * cch:512 * (cch + 1)],
                             in_=osb[cch % 2][:]).then_inc(sOD[cch % 2], 16)
            sy.wait_ge(sOD[0], 64)
            sy.wait_ge(sOD[1], 64)

        @block.gpsimd
        def _(gp):
            gp.wait_ge(sPSEND, 16)
            gp.collective_compute(
                "AllGather", ALU.bypass, replica_groups=GROUPS4,
                ins=[pool_send[:]], outs=[pool_gath[:]]).then_inc(sCC, 1)
            gp.wait_ge(sHSEND, 16)
            gp.collective_compute(
                "AllGather", ALU.bypass, replica_groups=GROUPS4,
                ins=[halo_send[:]], outs=[halo_gath[:]]).then_inc(sCC, 1)

        @block.tensor
        def _(pe):
            def prod_mm(tag, emit_mms):
                """Emit one u-bank production: WAR wait + matmuls; the
                last mm incs sUP."""
                p = P[tag]
                u_wait(pe, p)
                emit_mms(u[p % 2], p)

            pe.wait_ge(sXIN, 16)
            pe.wait_ge(sIN, 96)
            pe.wait_ge(sMS, 1)
            # q/k chunks: psum [2,512] <- lhsT [65,2] x xba-chunk [65,512]
            for cch in range(8):
                def mk_qk(col):
                    def f(ub, p, cch=cch, col=col):
                        pe.matmul(ub[0:2, :], c65_sb[:, col:col + 2],
                                  xba[:, 512 * cch:512 * (cch + 1)],
                                  start=True, stop=True).then_inc(sUP, 1)
                    return f
                prod_mm(("q", cch), mk_qk(0))
            for cch in range(8):
                def mk_k(ub, p, cch=cch):
                    pe.matmul(ub[0:2, :], c65_sb[:, 2:4],
                              xba[:, 512 * cch:512 * (cch + 1)],
                              start=True, stop=True).then_inc(sUP, 1)
                prod_mm(("k", cch), mk_k)
            # vT tiles: psum [128,65] <- lhsT xba-tile [65,128] x rhs65 [65,65]
            for t in range(MT):
                def mk_vt(ub, p, t=t):
                    pe.matmul(ub[0:128, 0:65], xba[:, 128 * t:128 * (t + 1)],
                              c65_sb[:, 8:73], start=True, stop=True).then_inc(sUP, 1)
                prod_mm(("vt", t), mk_vt)

            wait_consumed(pe, P[("k", 7)])

            # ---- gca / up / rb productions, interleaved into m-loops ----
            def emit_gca_0():   # after mt14 of nc0
                pe.wait_ge(sGIN, 16)

                def mk_gat(ub, p):
                    pe.matmul(ub[0:C, 0:4], gt_sb[:], c65_sb[0:4, 138:142],
                              start=True, stop=True).then_inc(sUP, 1)
                prod_mm(("gat",), mk_gat)

            def emit_gca_1():   # after mt16 of nc0
                wait_consumed(pe, P[("gat",)])
                pe.wait_ge(sMS, 2)

                def mk_vgt(ub, p):
                    pe.matmul(ub[0:4, 0:65], gaug_sb[:], c65_sb[:, 73:138],
                              start=True, stop=True).then_inc(sUP, 1)
                prod_mm(("vgt",), mk_vgt)

                def mk_gq(ub, p):
                    pe.matmul(ub[0:2, 0:4], c65_sb[:, 4:6], gaug_sb[:],
                              start=True, stop=True).then_inc(sUP, 1)
                prod_mm(("gq",), mk_gq)

                def mk_gk(ub, p):
                    pe.matmul(ub[0:2, 0:4], c65_sb[:, 6:8], gaug_sb[:],
                              start=True, stop=True).then_inc(sUP, 1)
                prod_mm(("gk",), mk_gk)

            def emit_gca_2():   # after mt18 of nc0
                pe.wait_ge(sGCAdve, 1)

                def mk_ltg(ub, p):
                    pe.matmul(ub[0:4, 0:4], kg_sb[:], qg_sb[:],
                              start=True, stop=True).then_inc(sUP, 1)
                prod_mm(("ltg",), mk_ltg)

            def emit_gca_3():   # after mt20 of nc0
                wait_consumed(pe, P[("ltg",)])   # etg_sb written

                def mk_outg(ub, p):
                    pe.matmul(ub[0:4, 0:65], etg_sb[:], vgt_sb[:],
                              start=True, stop=True).then_inc(sUP, 1)
                prod_mm(("outg",), mk_outg)

            def emit_up(cch):   # one chunk per call
                if cch == 0:
                    pe.wait_ge(sGCAdve, 2)

                def mk_up(ub, p):
                    pe.matmul(ub[0:C, :], gpt_sb[:],
                              mup_sb[:, 512 * cch:512 * (cch + 1)],
                              start=True, stop=True).then_inc(sUP, 1)
                prod_mm(("up", cch), mk_up)

            def emit_rb(nci):
                pe.wait_ge(sR, nci + 1)
                for h in range(2):
                    def mk_rb(ub, p, h=h):
                        pe.matmul(ub[0:C, :], grow_sb[:],
                                  r_sb[:, 512 * h:512 * (h + 1)],
                                  start=True, stop=True).then_inc(sUP, 1)
                    prod_mm(("rb", nci, h), mk_rb)

            hooks = {(0, 14): emit_gca_0,
                     (0, 16): emit_gca_1, (0, 18): emit_gca_2, (0, 20): emit_gca_3}
            for _c in range(8):
                hooks[(0, 22 + _c)] = (lambda cch: lambda: emit_up(cch))(_c)

            def emit_outp(g):
                mt_o, nci_o = g % MT, g // MT
                if mt_o == 0:
                    if g == 0:
                        wait_consumed(pe, P[("vt", MT - 1)])
                    if nci_o >= 1:
                        emit_rb(nci_o - 1)
                        pe.wait_ge(sEPo, nci_o)
                pe.wait_ge(sE, g + 1)
                off = CW * nci_o
                st, sp = (mt_o == 0), (mt_o == MT - 1)
                pe.matmul(po[0:C + 1, 0:512], vt_sb[:, 65 * mt_o:65 * mt_o + 65],
                          et[g % 2][:, 0:512], start=st, stop=sp)
                pe.matmul(po[0:C + 1, 512:1024], vt_sb[:, 65 * mt_o:65 * mt_o + 65],
                          et[g % 2][:, 512:1024], start=st, stop=sp).then_inc(sOA, 1)

            for nci in range(NCH):
                for mt in range(MT):
                    g = nci * MT + mt
                    # L production for (nci, mt)
                    if g >= 2:
                        pe.wait_ge(sE, g - 1)
                    pe.matmul(lt[g % 2][:, 0:512], k_sb[:, 128 * mt:128 * (mt + 1)],
                              q_sb[:, CW * nci:CW * nci + 512], start=True, stop=True)
                    pe.matmul(lt[g % 2][:, 512:1024], k_sb[:, 128 * mt:128 * (mt + 1)],
                              q_sb[:, CW * nci + 512:CW * nci + 1024],
                              start=True, stop=True).then_inc(sL, 1)
                    if g >= 1:
                        emit_outp(g - 1)
                    if (nci, mt) in hooks:
                        hooks[(nci, mt)]()
            emit_outp(NCH * MT - 1)
            emit_rb(3)

            # ---- conv ----
            pe.wait_ge(sHALO, 1)
            pe.wait_ge(sGATED, NCH)
            for cch in range(8):
                def mk_conv(ub, p, cch=cch):
                    kidx = 0
                    for ky in range(3):
                        for kx in range(3):
                            mm = pe.matmul(
                                ub[0:C, :], wconv_sb[:, 64 * (3 * ky + kx):64 * (3 * ky + kx) + 64],
                                xc[:, 8 * cch + ky:8 * cch + ky + 8, kx:kx + WB],
                                start=(kidx == 0), stop=(kidx == 8))
                            kidx += 1
                    mm.then_inc(sUP, 1)
                prod_mm(("conv", cch), mk_conv)

        @block.scalar
        def _(act):
            # q/k psum -> sbuf copies
            for tag, dst in [("q", q_sb), ("k", k_sb)]:
                for cch in range(8):
                    p = P[(tag, cch)]
                    act.wait_ge(sUP, p + 1)
                    act.activation(dst[:, 512 * cch:512 * (cch + 1)], u[p % 2][0:2, :],
                                   AF.Copy).then_inc(uc[p % 2], 1)

            def emit_etg():
                p = P[("ltg",)]
                act.wait_ge(sUP, p + 1)
                act.activation(etg_sb[:], u[p % 2][0:4, 0:4],
                               AF.Exp).then_inc(uc[p % 2], 1)

            def emit_rg():
                # rg = 1/zg via exp(-ln(zg))
                act.wait_ge(sZG, 1)
                act.activation(rg_sb[:], zg_sb[:], AF.Ln)
                act.drain()
                act.activation(rg_sb[:], rg_sb[:], AF.Exp,
                               scale=-1.0).then_inc(sRG, 1)

            def emit_esig(cch):
                # sig = 1/(1+exp(-up)) step 1: e = exp(-up), one chunk
                p = P[("up", cch)]
                act.wait_ge(sUP, p + 1)
                act.activation(e_sb[:, 512 * cch:512 * (cch + 1)], u[p % 2][0:C, :],
                               AF.Exp, scale=-1.0).then_inc(uc[p % 2], 1)

            def emit_sig():
                act.drain()
                act.activation(scr_sb[:], e_sb[:], AF.Ln, bias=1.0)
                act.drain()
                act.activation(sig_sb[:], scr_sb[:], AF.Exp,
                               scale=-1.0).then_inc(sSD, 1)

            def emit_r(nci):
                # r = 1/Z via exp(-ln(Z)); Z = po row 64
                act.wait_ge(sOA, MT * (nci + 1))
                if nci >= 1:
                    # WAR: PE's RB(nci-1) must have read r_sb
                    act.wait_ge(sUP, P[("rb", nci - 1, 1)] + 1)
                act.drain()
                act.activation(r_sb[:], po[C:C + 1, :], AF.Ln)
                act.drain()
                act.activation(r_sb[:], r_sb[:], AF.Exp,
                               scale=-1.0).then_inc(sR, 1)

            for g in range(NCH * MT):
                act.wait_ge(sL, g + 1)
                if g >= 2:
                    act.wait_ge(sOA, g - 1)
                act.activation(et[g % 2][:], lt[g % 2][:], AF.Exp).then_inc(sE, 1)
                if g == 20:
                    emit_etg()
                elif g == 22:
                    emit_rg()
                elif 24 <= g <= 31:
                    emit_esig(g - 24)
                if g % MT == MT - 1:
                    emit_r(g // MT)
                if g == 31:
                    emit_sig()

            # conv relu
            for cch in range(8):
                act.wait_ge(sT2, cch + 1)
                if cch >= 2:
                    act.wait_ge(sOD[cch % 2], 16 * (cch // 2))
                act.activation(osb[cch % 2][:], t2[cch % 2][:],
                               AF.Relu).then_inc(sOUT, 1)

        @block.vector
        def _(dve):
            dve.wait_ge(sXIN, 16)
            dve.tensor_reduce(pooled_sb[:], xba[0:C, :], axis=AX.X,
                              op=ALU.max).then_inc(sPOOL, 1)
            dve.memset(xba[C:C + 1, :], 1.0).then_inc(sMS, 1)
            dve.memset(gaug_sb[C:C + 1, :], 1.0).then_inc(sMS, 1)
            dve.memset(xc[:], 0.0)
            # vT copies
            for t in range(MT):
                p = P[("vt", t)]
                dve.wait_ge(sUP, p + 1)
                dve.tensor_copy(vt_sb[:, 65 * t:65 * (t + 1)],
                                u[p % 2][0:128, 0:65]).then_inc(uc[p % 2], 1)
            # gca small ops
            p = P[("gat",)]
            dve.wait_ge(sUP, p + 1)
            dve.tensor_copy(gaug_sb[0:C, :],
                            u[p % 2][0:C, 0:4]).then_inc(uc[p % 2], 1)
            for tag, dst, rows, cols in [("vgt", vgt_sb, 4, 65),
                                         ("gq", qg_sb, 2, 4), ("gk", kg_sb, 2, 4)]:
                p = P[(tag,)]
                dve.wait_ge(sUP, p + 1)
                dve.tensor_copy(dst[:], u[p % 2][0:rows, 0:cols]).then_inc(uc[p % 2], 1)
                if tag == "gk":
                    dve.drain()
                    dve.memset(hB[0:1, 0:1], 0.0).then_inc(sGCAdve, 1)   # -> 1
            p = P[("outg",)]
            dve.wait_ge(sUP, p + 1)
            dve.tensor_copy(numt_sb[:], u[p % 2][0:4, 0:C])
            dve.tensor_copy(zg_sb[:], u[p % 2][0:4, C:C + 1]).then_inc(uc[p % 2], 1)
            dve.drain()
            dve.memset(hB[0:1, 0:1], 0.0).then_inc(sZG, 1)
            dve.wait_ge(sRG, 1)
            dve.tensor_scalar(gtmp_sb[:], numt_sb[:], rg_sb[:], gca_gamma,
                              op0=ALU.mult, op1=ALU.mult)
            dve.drain()
            dve.tensor_tensor(gpt_sb[:], gtmp_sb[:], gt_sb[:],
                              op=ALU.add).then_inc(sGCAdve, 1)   # -> 2
            # p1 = sig * x (sig computed on ACT)
            dve.wait_ge(sSD, 1)
            dve.tensor_tensor(p1_sb[:], sig_sb[:], xba[0:C, :], op=ALU.mult)
            # epilogues
            for nci in range(NCH):
                off = CW * nci
                # r computed on ACT (sR); S2 below waits on rb prods
                for h in range(2):
                    p = P[("rb", nci, h)]
                    dve.wait_ge(sUP, p + 1)
                    dve.tensor_tensor(s2_sb[:, 512 * h:512 * (h + 1)],
                                      sig_sb[:, off + 512 * h:off + 512 * (h + 1)],
                                      u[p % 2][0:C, :], op=ALU.mult).then_inc(uc[p % 2], 1)
                dve.drain()
                dve.tensor_tensor(g_sb[:], po[0:C, :], s2_sb[:],
                                  op=ALU.mult).then_inc(sEPo, 1)
                dve.drain()
                dve.tensor_tensor(xc[:, 1 + 16 * nci:1 + 16 * (nci + 1), 1:WB + 1],
                                  g_sb[:], p1_sb[:, off:off + CW],
                                  op=ALU.add).then_inc(sGATED, 1)
            # pack halo send strip (col 63, row 63, corner of gated tile)
            dve.drain()
            dve.tensor_copy(hsend_sb[:, 0:WB], xc[:, 1:HB + 1, WB])
            dve.tensor_copy(hsend_sb[:, WB:2 * WB], xc[:, HB, 1:WB + 1])
            dve.drain()
            dve.tensor_copy(hsend_sb[:, 2 * WB:2 * WB + 1],
                            xc[:, HB, WB:WB + 1]).then_inc(sHS, 1)
            # halo merge (mask-select the 3 neighbor strips)
            dve.wait_ge(sHG, 16)
            dve.wait_ge(sIN, 96)
            dve.tensor_tensor(hA[:], hg_sb[:, 0, :], hmask_sb[:, 0:129], op=ALU.mult)
            dve.tensor_tensor(hB[:], hg_sb[:, 1, :], hmask_sb[:, 129:258], op=ALU.mult)
            dve.tensor_tensor(hC[:], hg_sb[:, 2, :], hmask_sb[:, 258:387], op=ALU.mult)
            dve.tensor_tensor(hD[:], hg_sb[:, 3, :], hmask_sb[:, 387:516], op=ALU.mult)
            dve.drain()
            dve.tensor_tensor(hA[:], hA[:], hB[:], op=ALU.add)
            dve.tensor_tensor(hC[:], hC[:], hD[:], op=ALU.add)
            dve.drain()
            dve.tensor_tensor(hA[:], hA[:], hC[:], op=ALU.add)
            dve.drain()
            dve.tensor_copy(xc[:, 1:HB + 1, WB + 1], hA[:, 0:WB])
            dve.tensor_copy(xc[:, HB + 1, 1:WB + 1], hA[:, WB:2 * WB])
            dve.drain()
            dve.tensor_copy(xc[:, HB + 1, WB + 1:WB + 2],
                            hA[:, 2 * WB:2 * WB + 1]).then_inc(sHALO, 1)
            # conv epilogue
            for cch in range(8):
                p = P[("conv", cch)]
                dve.wait_ge(sUP, p + 1)
                dve.tensor_scalar(tA[cch % 2][:], u[p % 2][0:C, :], gamma, b2_sb[:],
                                  op0=ALU.mult, op1=ALU.add).then_inc(uc[p % 2], 1)
                dve.drain()
                if cch >= 2:
                    dve.wait_ge(sOUT, cch - 1)
                dve.tensor_tensor(t2[cch % 2][:], tA[cch % 2][:],
                                  xba[0:C, 512 * cch:512 * (cch + 1)],
                                  op=ALU.add).then_inc(sT2, 1)

    return nc, ctx


_CACHE = {}


def kernel(**inputs):
    in_maps, sc = prep_inputs(inputs)
    key = (sc['nl_gamma'], sc['gca_gamma'], sc['gamma'])
    if key not in _CACHE:
        _CACHE[key] = build_nc(**sc)
    nc, _ctx = _CACHE[key]
    res = run_bass_kernel_spmd(nc, in_maps, core_ids=list(range(8)))
    outs = [res.results[i]["out"] for i in range(8)]
    return unshard(outs).astype(np.float32)


if __name__ == "__main__":
    nc, _ = build_nc(0.1, 0.1, 0.1)
    print("built ok;", len(nc.m.functions[0].allocations), "allocations")


# revision 17
# speedup vs baseline: 2.4111x; 2.4111x over previous
"""Trainium2 Bass kernel for nn_AGCB_Element (sparse_attention).

Sharding: pure data parallel over (batch=2) x (2x2 spatial blocks) = 8
cores; one (batch, block) non-local attention unit per core, fully
SBUF/PSUM-resident. Params replicated. Two tiny AllGathers per batch
group of 4 cores: pooled 2x2 maxima (for the GCA branch, computed
redundantly per group) and gated-context halo edges (for the 3x3 conv).

SPMD uniformity: all cores run one graph, so per-core spatial geometry
is normalized by flipping x/y of the inputs on the host (conv weights,
upsample matrix, x tile flipped as data; outputs unflipped). Halo
neighbor selection uses per-core 0/1 mask input tensors.

Attention per core (N=4096, inter=2), transposed-layout softmax:
  Lt[m,n] = k^T q;  Et = exp(Lt)  (no max subtraction: |Lt| < ~14);
  out' = [v; 1]^T Et  -> row 64 is the denominator Z[n];
  ctx = num * (sig * nl_gamma / Z) + sig * x.

Raw bass (explicit engines/semaphores) - the Tile framework emits
multi-wait instructions this walrus build rejects.
"""
import sys

if "/opt/trn_rl_repo" not in sys.path:
    sys.path.insert(0, "/opt/trn_rl_repo")

from contextlib import ExitStack

import numpy as np
import ml_dtypes

import concourse.bass as bass
import concourse.mybir as mybir
from concourse.bass_utils import run_bass_kernel_spmd

C = 64
HB = WB = 64
N = HB * WB            # 4096 spatial positions per block
NCH = 4                # n-chunks
CW = N // NCH          # 1024
MT = 32                # m-tiles of 128
EPS = 1e-5
F32 = mybir.dt.float32
BF16 = mybir.dt.bfloat16
AF = mybir.ActivationFunctionType
ALU = mybir.AluOpType
AX = mybir.AxisListType
GROUPS4 = [[0, 1, 2, 3], [4, 5, 6, 7]]


def _interp_w(n_out, n_in=2):
    ys = np.linspace(0.0, n_in - 1.0, n_out)
    y0 = np.clip(np.floor(ys).astype(np.int64), 0, n_in - 1)
    y1 = np.minimum(y0 + 1, n_in - 1)
    wy = ys - y0
    W = np.zeros((n_out, n_in), np.float64)
    for r in range(n_out):
        W[r, y0[r]] += 1.0 - wy[r]
        W[r, y1[r]] += wy[r]
    return W.astype(np.float32)


def prep_inputs(inputs):
    """Host-side sharding + parameter prep. Returns (in_maps, scalars)."""
    f32 = np.float32
    x = np.asarray(inputs['x'])
    c65 = np.zeros((C + 1, 142), f32)
    c65[0:4, 138:142] = np.eye(4, dtype=f32)
    c65[:, 0:2] = np.concatenate([np.asarray(inputs['nl_q_w']).T,
                                  np.asarray(inputs['nl_q_b'])[None, :]], 0)
    c65[:, 2:4] = np.concatenate([np.asarray(inputs['nl_k_w']).T,
                                  np.asarray(inputs['nl_k_b'])[None, :]], 0)
    c65[:, 4:6] = np.concatenate([np.asarray(inputs['gca_q_w']).T,
                                  np.asarray(inputs['gca_q_b'])[None, :]], 0)
    c65[:, 6:8] = np.concatenate([np.asarray(inputs['gca_k_w']).T,
                                  np.asarray(inputs['gca_k_b'])[None, :]], 0)
    rhs65 = np.zeros((C + 1, C + 1), f32)
    rhs65[:C, :C] = np.asarray(inputs['nl_v_w']).T
    rhs65[C, :C] = np.asarray(inputs['nl_v_b'])
    rhs65[C, C] = 1.0
    c65[:, 8:73] = rhs65
    grhs65 = np.zeros((C + 1, C + 1), f32)
    grhs65[:C, :C] = np.asarray(inputs['gca_v_w']).T
    grhs65[C, :C] = np.asarray(inputs['gca_v_b'])
    grhs65[C, C] = 1.0
    c65[:, 73:138] = grhs65

    nl_gamma = float(inputs['nl_gamma'])
    gca_gamma = float(inputs['gca_gamma'])
    gamma = float(inputs['gamma'])
    scale = np.asarray(inputs['bn_w']) / np.sqrt(np.asarray(inputs['bn_var']) + EPS)
    Wc = np.asarray(inputs['conv_w']) * scale[:, None, None, None]
    bc = ((np.asarray(inputs['conv_b']) - np.asarray(inputs['bn_mean'])) * scale
          + np.asarray(inputs['bn_b']))
    b2 = (gamma * bc).astype(f32).reshape(C, 1)
    grow = np.full((1, C), nl_gamma, f32)
    Wy = _interp_w(2 * HB)
    Wx = _interp_w(2 * WB)

    in_maps = []
    for core in range(8):
        b, blk = core // 4, core % 4
        i0, j0 = blk // 2, blk % 2
        fy, fx = (i0 == 1), (j0 == 1)
        xt = x[b, :, i0 * HB:(i0 + 1) * HB, j0 * WB:(j0 + 1) * WB]
        if fy:
            xt = xt[:, ::-1, :]
        if fx:
            xt = xt[:, :, ::-1]
        xt = np.ascontiguousarray(xt).reshape(C, N).astype(f32)
        Wcf = Wc
        if fy:
            Wcf = Wcf[:, :, ::-1, :]
        if fx:
            Wcf = Wcf[:, :, :, ::-1]
        wconv = np.ascontiguousarray(Wcf.transpose(1, 2, 3, 0)).reshape(C, 9 * C).astype(f32)
        Wy_t = Wy[i0 * HB:(i0 + 1) * HB]
        Wx_t = Wx[j0 * WB:(j0 + 1) * WB]
        if fy:
            Wy_t = Wy_t[::-1]
        if fx:
            Wx_t = Wx_t[::-1]
        m_up = np.einsum('pi,qj->ijpq', Wy_t, Wx_t).reshape(4, N).astype(f32)
        r_h, r_v, r_d = blk ^ 1, blk ^ 2, blk ^ 3
        hmask = np.zeros((C, 4, 129), f32)
        hmask[:, r_h, 0:WB] = 1.0
        hmask[:, r_v, WB:2 * WB] = 1.0
        hmask[:, r_d, 2 * WB] = 1.0
        bf = ml_dtypes.bfloat16
        in_maps.append(dict(
            x_tile=xt, c65=c65, grow=grow.astype(bf), b2=b2,
            m_up=m_up.astype(bf), wconv=wconv.astype(bf),
            hmask=np.ascontiguousarray(hmask.reshape(C, 4 * 129)).astype(bf)))
    return in_maps, dict(nl_gamma=nl_gamma, gca_gamma=gca_gamma, gamma=gamma)


def unshard(outs):
    f32 = np.float32
    out = np.zeros((2, C, 2 * HB, 2 * WB), f32)
    for core in range(8):
        b, blk = core // 4, core % 4
        i0, j0 = blk // 2, blk % 2
        t = np.asarray(outs[core]).reshape(C, HB, WB)
        if i0 == 1:
            t = t[:, ::-1, :]
        if j0 == 1:
            t = t[:, :, ::-1]
        out[b, :, i0 * HB:(i0 + 1) * HB, j0 * WB:(j0 + 1) * WB] = t
    return out


def build_nc(nl_gamma, gca_gamma, gamma):
    nc = bass.Bass(num_devices=8)
    ctx = ExitStack()

    x_ext = nc.declare_dram_parameter("x_tile", [C, N], F32, isOutput=False)
    c65_ext = nc.declare_dram_parameter("c65", [C + 1, 142], F32, isOutput=False)
    grow_ext = nc.declare_dram_parameter("grow", [1, C], BF16, isOutput=False)
    b2_ext = nc.declare_dram_parameter("b2", [C, 1], F32, isOutput=False)
    mup_ext = nc.declare_dram_parameter("m_up", [4, N], BF16, isOutput=False)
    wconv_ext = nc.declare_dram_parameter("wconv", [C, 9 * C], BF16, isOutput=False)
    hmask_ext = nc.declare_dram_parameter("hmask", [C, 4 * 129], BF16, isOutput=False)
    out_ext = nc.declare_dram_parameter("out", [C, N], F32, isOutput=True)

    pool_send = nc.dram_tensor("pool_send", [C], F32)
    pool_gath = nc.dram_tensor("pool_gath", [4, C], F32)
    halo_send = nc.dram_tensor("halo_send", [C, 129], BF16)
    halo_gath = nc.dram_tensor("halo_gath", [4 * C, 129], BF16)

    # ---- u-bank production schedule (PE order) ----
    prods = []
    prods += [("q", c) for c in range(8)]
    prods += [("k", c) for c in range(8)]
    prods += [("vt", t) for t in range(MT)]
    prods += [("gat",), ("vgt",), ("gq",), ("gk",), ("ltg",), ("outg",)]
    prods += [("up", c) for c in range(8)]
    for nci in range(NCH):
        prods += [("rb", nci, 0), ("rb", nci, 1)]
    prods += [("conv", c) for c in range(8)]
    P = {p: i for i, p in enumerate(prods)}
    NCONV0 = P[("conv", 0)]

    _names = [0]

    def sb(shape, name=None, dt=F32):
        _names[0] += 1
        return ctx.enter_context(nc.sbuf_tensor(name or f"sb{_names[0]}", shape, dt))

    def ps(shape):
        _names[0] += 1
        return ctx.enter_context(nc.psum_tensor(f"ps{_names[0]}", shape, F32))

    sem = lambda name: ctx.enter_context(nc.semaphore(name))

    xba = sb([C + 1, N])
    q_sb = sb([2, N], dt=BF16)
    k_sb = sb([2, N], dt=BF16)
    vt_sb = sb([128, MT * 65], dt=BF16)
    et = [sb([128, CW], dt=BF16), sb([128, CW], dt=BF16)]
    xba_bf = sb([C + 1, N], dt=BF16)
    c65_sb = sb([C + 1, 142])
    c65b_sb = sb([C + 1, 73], dt=BF16)
    grow_sb = sb([1, C], dt=BF16)
    b2_sb = sb([C, 1])
    mup_sb = sb([4, N], dt=BF16)
    wconv_sb = sb([C, 9 * C], dt=BF16)
    hmask_sb = sb([C, 4 * 129], dt=BF16)
    pooled_sb = sb([C, 1])
    gaug_sb = sb([C + 1, 4])
    gt_sb = sb([4, C])
    qg_sb = sb([2, 4])
    kg_sb = sb([2, 4])
    etg_sb = sb([4, 4])
    vgt_sb = sb([4, 65])
    numt_sb = sb([4, C])
    rg_sb = sb([4, 1])
    gtmp_sb = sb([4, C])
    gpt_sb = sb([4, C], dt=BF16)
    e_sb = sb([C, N])
    sig_sb = sb([C, N])
    scr_sb = sb([C, N])
    p1_sb = sb([C, N])
    rln_sb = sb([1, CW])
    r_sb = sb([1, CW], dt=BF16)
    zg_sb = sb([4, 1])
    s2_sb = sb([C, CW])
    g_sb = sb([C, CW])
    xc = sb([C, HB + 2, WB + 2], dt=BF16)
    hg_sb = sb([C, 4, 129], dt=BF16)
    hsend_sb = sb([C, 129], dt=BF16)
    hA = sb([C, 129], dt=BF16)
    hB = sb([C, 129], dt=BF16)
    hC = sb([C, 129], dt=BF16)
    hD = sb([C, 129], dt=BF16)
    tA = [sb([C, 512]), sb([C, 512])]
    t2 = [sb([C, 512]), sb([C, 512])]
    osb = [sb([C, 512]), sb([C, 512])]

    lt = [ps([128, CW]), ps([128, CW])]   # psum banks 0-1, 2-3
    po = ps([128, CW])                    # banks 4-5
    u = [ps([128, 512]), ps([128, 512])]  # banks 6, 7

    sXIN = sem("sXIN")      # x_tile DMA (16)
    sIN = sem("sIN")        # const input DMAs (6 x 16 = 96)
    sXBF = sem("sXBF")      # ACT bf16 casts of xba + c65 (2)
    sMS = sem("sMS")        # DVE memsets: xba ones (1), gaug ones (2)
    sPOOL = sem("sPOOL")
    sPSEND = sem("sPSEND")
    sCC = sem("sCC")        # collectives: pool AG (1), halo AG (2)
    sGIN = sem("sGIN")      # gaug+gt DMAs (32)
    sL = sem("sL")          # PE L-pair per mt
    sE = sem("sE")          # ACT exp per mt
    sOA = sem("sOA")        # PE out'-pair per mt
    sR = sem("sR")          # DVE r ready (per nci)
    sEPo = sem("sEPo")      # DVE done reading po (per nci)
    sGATED = sem("sGATED")  # DVE gated chunk written (per nci)
    sUP = sem("sUP")        # PE u-bank production counter
    uc = [sem("u0c"), sem("u1c")]  # per-u-bank consumption counters
    sGCAdve = sem("sGCAdve")
    sZG = sem("sZG")
    sRG = sem("sRG")
    sSD = sem("sSD")
    sHS = sem("sHS")        # DVE packed halo-send strip (1)
    sHSEND = sem("sHSEND")  # halo-send DMA (16)
    sHG = sem("sHG")        # halo_gath -> sbuf DMA (16)
    sHALO = sem("sHALO")    # DVE halo merged into xc (1)
    sOUT = sem("sOUT")      # ACT relu per chunk
    sOD = [sem("sOD0"), sem("sOD1")]  # out DMAs by chunk parity

    def u_wait(eng, p):
        if p >= 2:
            eng.wait_ge(uc[p % 2], p // 2)

    def cons_through(p):
        # (bank0, bank1) consumption counts once productions 0..p are all
        # consumed
        n0 = sum(1 for i in range(p + 1) if i % 2 == 0)
        return n0, (p + 1) - n0

    def wait_consumed(eng, p):
        n0, n1 = cons_through(p)
        eng.wait_ge(uc[0], n0)
        eng.wait_ge(uc[1], n1)

    # ---------------- engine programs ----------------
    with nc.Block() as block:

        @block.sync
        def _(sy):
            sy.dma_start(out=xba[0:C, :], in_=x_ext[:]).then_inc(sXIN, 16)
            sy.dma_start(out=c65_sb[:], in_=c65_ext[:]).then_inc(sIN, 16)
            sy.dma_start(out=grow_sb[:], in_=grow_ext[:]).then_inc(sIN, 16)
            sy.dma_start(out=b2_sb[:], in_=b2_ext[:]).then_inc(sIN, 16)
            sy.dma_start(out=mup_sb[:], in_=mup_ext[:]).then_inc(sIN, 16)
            sy.dma_start(out=wconv_sb[:], in_=wconv_ext[:]).then_inc(sIN, 16)
            sy.dma_start(out=hmask_sb[:], in_=hmask_ext[:]).then_inc(sIN, 16)
            # pooled maxima out
            sy.wait_ge(sPOOL, 1)
            sy.dma_start(out=pool_send[:], in_=pooled_sb[:, 0:1]).then_inc(sPSEND, 16)
            # gathered pool back in (transposed into [c, n] + direct [n, c])
            sy.wait_ge(sCC, 1)
            sy.dma_start(out=gt_sb[:], in_=pool_gath[:]).then_inc(sGIN, 16)
            # halo send (uniform: local col 63, row 63, corner),
            # packed contiguously by DVE first
            sy.wait_ge(sHS, 1)
            sy.dma_start(out=halo_send[:], in_=hsend_sb[:]).then_inc(sHSEND, 16)
            # gathered halo in
            sy.wait_ge(sCC, 2)
            sy.dma_start(out=hg_sb[:],
                         in_=halo_gath[:].rearrange("(s c) j -> c s j", s=4)).then_inc(sHG, 16)
            # outputs
            for cch in range(8):
                sy.wait_ge(sOUT, cch + 1)
                sy.dma_start(out=out_ext[:, 512 * cch:512 * (cch + 1)],
                             in_=osb[cch % 2][:]).then_inc(sOD[cch % 2], 16)
            sy.wait_ge(sOD[0], 64)
            sy.wait_ge(sOD[1], 64)

        @block.gpsimd
        def _(gp):
            gp.wait_ge(sPSEND, 16)
            gp.collective_compute(
                "AllGather", ALU.bypass, replica_groups=GROUPS4,
                ins=[pool_send[:]], outs=[pool_gath[:]]).then_inc(sCC, 1)
            gp.wait_ge(sHSEND, 16)
            gp.collective_compute(
                "AllGather", ALU.bypass, replica_groups=GROUPS4,
                ins=[halo_send[:]], outs=[halo_gath[:]]).then_inc(sCC, 1)

        @block.tensor
        def _(pe):
            def prod_mm(tag, emit_mms):
                """Emit one u-bank production: WAR wait + matmuls; the
                last mm incs sUP."""
                p = P[tag]
                u_wait(pe, p)
                emit_mms(u[p % 2], p)

            pe.wait_ge(sXIN, 16)
            pe.wait_ge(sIN, 96)
            pe.wait_ge(sXBF, 2)
            pe.wait_ge(sMS, 1)
            # q/k chunks: psum [2,512] <- lhsT [65,2] x xba-chunk [65,512]
            for cch in range(8):
                def mk_qk(col):
                    def f(ub, p, cch=cch, col=col):
                        pe.matmul(ub[0:2, :], c65b_sb[:, col:col + 2],
                                  xba_bf[:, 512 * cch:512 * (cch + 1)],
                                  start=True, stop=True).then_inc(sUP, 1)
                    return f
                prod_mm(("q", cch), mk_qk(0))
            for cch in range(8):
                def mk_k(ub, p, cch=cch):
                    pe.matmul(ub[0:2, :], c65b_sb[:, 2:4],
                              xba_bf[:, 512 * cch:512 * (cch + 1)],
                              start=True, stop=True).then_inc(sUP, 1)
                prod_mm(("k", cch), mk_k)
            # vT tiles: psum [128,65] <- lhsT xba-tile [65,128] x rhs65 [65,65]
            for t in range(MT):
                def mk_vt(ub, p, t=t):
                    pe.matmul(ub[0:128, 0:65], xba_bf[:, 128 * t:128 * (t + 1)],
                              c65b_sb[:, 4:69], start=True, stop=True).then_inc(sUP, 1)
                prod_mm(("vt", t), mk_vt)

            wait_consumed(pe, P[("k", 7)])

            # ---- gca / up / rb productions, interleaved into m-loops ----
            def emit_gca_0():   # after mt14 of nc0
                pe.wait_ge(sGIN, 16)

                def mk_gat(ub, p):
                    pe.matmul(ub[0:C, 0:4], gt_sb[:], c65_sb[0:4, 138:142],
                              start=True, stop=True).then_inc(sUP, 1)
                prod_mm(("gat",), mk_gat)

            def emit_gca_1():   # after mt16 of nc0
                wait_consumed(pe, P[("gat",)])
                pe.wait_ge(sMS, 2)

                def mk_vgt(ub, p):
                    pe.matmul(ub[0:4, 0:65], gaug_sb[:], c65_sb[:, 73:138],
                              start=True, stop=True).then_inc(sUP, 1)
                prod_mm(("vgt",), mk_vgt)

                def mk_gq(ub, p):
                    pe.matmul(ub[0:2, 0:4], c65_sb[:, 4:6], gaug_sb[:],
                              start=True, stop=True).then_inc(sUP, 1)
                prod_mm(("gq",), mk_gq)

                def mk_gk(ub, p):
                    pe.matmul(ub[0:2, 0:4], c65_sb[:, 6:8], gaug_sb[:],
                              start=True, stop=True).then_inc(sUP, 1)
                prod_mm(("gk",), mk_gk)

            def emit_gca_2():   # after mt18 of nc0
                pe.wait_ge(sGCAdve, 1)

                def mk_ltg(ub, p):
                    pe.matmul(ub[0:4, 0:4], kg_sb[:], qg_sb[:],
                              start=True, stop=True).then_inc(sUP, 1)
                prod_mm(("ltg",), mk_ltg)

            def emit_gca_3():   # after mt20 of nc0
                wait_consumed(pe, P[("ltg",)])   # etg_sb written

                def mk_outg(ub, p):
                    pe.matmul(ub[0:4, 0:65], etg_sb[:], vgt_sb[:],
                              start=True, stop=True).then_inc(sUP, 1)
                prod_mm(("outg",), mk_outg)

            def emit_up(cch):   # one chunk per call
                if cch == 0:
                    pe.wait_ge(sGCAdve, 2)

                def mk_up(ub, p):
                    pe.matmul(ub[0:C, :], gpt_sb[:],
                              mup_sb[:, 512 * cch:512 * (cch + 1)],
                              start=True, stop=True).then_inc(sUP, 1)
                prod_mm(("up", cch), mk_up)

            def emit_rb(nci):
                pe.wait_ge(sR, nci + 1)
                for h in range(2):
                    def mk_rb(ub, p, h=h):
                        pe.matmul(ub[0:C, :], grow_sb[:],
                                  r_sb[:, 512 * h:512 * (h + 1)],
                                  start=True, stop=True).then_inc(sUP, 1)
                    prod_mm(("rb", nci, h), mk_rb)

            hooks = {(0, 14): emit_gca_0,
                     (0, 16): emit_gca_1, (0, 18): emit_gca_2, (0, 20): emit_gca_3}
            for _c in range(8):
                hooks[(0, 22 + _c)] = (lambda cch: lambda: emit_up(cch))(_c)

            def emit_outp(g):
                mt_o, nci_o = g % MT, g // MT
                if mt_o == 0:
                    if g == 0:
                        wait_consumed(pe, P[("vt", MT - 1)])
                    if nci_o >= 1:
                        emit_rb(nci_o - 1)
                        pe.wait_ge(sEPo, nci_o)
                pe.wait_ge(sE, g + 1)
                off = CW * nci_o
                st, sp = (mt_o == 0), (mt_o == MT - 1)
                pe.matmul(po[0:C + 1, 0:512], vt_sb[:, 65 * mt_o:65 * mt_o + 65],
                          et[g % 2][:, 0:512], start=st, stop=sp)
                pe.matmul(po[0:C + 1, 512:1024], vt_sb[:, 65 * mt_o:65 * mt_o + 65],
                          et[g % 2][:, 512:1024], start=st, stop=sp).then_inc(sOA, 1)

            for nci in range(NCH):
                for mt in range(MT):
                    g = nci * MT + mt
                    # L production for (nci, mt)
                    if g >= 2:
                        pe.wait_ge(sE, g - 1)
                    pe.matmul(lt[g % 2][:, 0:512], k_sb[:, 128 * mt:128 * (mt + 1)],
                              q_sb[:, CW * nci:CW * nci + 512], start=True, stop=True)
                    pe.matmul(lt[g % 2][:, 512:1024], k_sb[:, 128 * mt:128 * (mt + 1)],
                              q_sb[:, CW * nci + 512:CW * nci + 1024],
                              start=True, stop=True).then_inc(sL, 1)
                    if g >= 1:
                        emit_outp(g - 1)
                    if (nci, mt) in hooks:
                        hooks[(nci, mt)]()
            emit_outp(NCH * MT - 1)
            emit_rb(3)

            # ---- conv ----
            pe.wait_ge(sHALO, 1)
            pe.wait_ge(sGATED, NCH)
            for cch in range(8):
                def mk_conv(ub, p, cch=cch):
                    kidx = 0
                    for ky in range(3):
                        for kx in range(3):
                            mm = pe.matmul(
                                ub[0:C, :], wconv_sb[:, 64 * (3 * ky + kx):64 * (3 * ky + kx) + 64],
                                xc[:, 8 * cch + ky:8 * cch + ky + 8, kx:kx + WB],
                                start=(kidx == 0), stop=(kidx == 8))
                            kidx += 1
                    mm.then_inc(sUP, 1)
                prod_mm(("conv", cch), mk_conv)

        @block.scalar
        def _(act):
            # bf16 casts for PE operands
            act.wait_ge(sXIN, 16)
            act.wait_ge(sMS, 1)
            act.activation(xba_bf[:], xba[:], AF.Copy).then_inc(sXBF, 1)
            act.wait_ge(sIN, 96)
            act.activation(c65b_sb[:, 0:4], c65_sb[:, 0:4], AF.Copy)
            act.activation(c65b_sb[:, 4:69], c65_sb[:, 8:73],
                           AF.Copy).then_inc(sXBF, 1)
            # q/k psum -> sbuf copies
            for tag, dst in [("q", q_sb), ("k", k_sb)]:
                for cch in range(8):
                    p = P[(tag, cch)]
                    act.wait_ge(sUP, p + 1)
                    act.activation(dst[:, 512 * cch:512 * (cch + 1)], u[p % 2][0:2, :],
                                   AF.Copy).then_inc(uc[p % 2], 1)

            def emit_etg():
                p = P[("ltg",)]
                act.wait_ge(sUP, p + 1)
                act.activation(etg_sb[:], u[p % 2][0:4, 0:4],
                               AF.Exp).then_inc(uc[p % 2], 1)

            def emit_rg():
                # rg = 1/zg via exp(-ln(zg))
                act.wait_ge(sZG, 1)
                act.activation(rg_sb[:], zg_sb[:], AF.Ln)
                act.drain()
                act.activation(rg_sb[:], rg_sb[:], AF.Exp,
                               scale=-1.0).then_inc(sRG, 1)

            def emit_esig(cch):
                # sig = 1/(1+exp(-up)) step 1: e = exp(-up), one chunk
                p = P[("up", cch)]
                act.wait_ge(sUP, p + 1)
                act.activation(e_sb[:, 512 * cch:512 * (cch + 1)], u[p % 2][0:C, :],
                               AF.Exp, scale=-1.0).then_inc(uc[p % 2], 1)

            def emit_sig():
                act.drain()
                act.activation(scr_sb[:], e_sb[:], AF.Ln, bias=1.0)
                act.drain()
                act.activation(sig_sb[:], scr_sb[:], AF.Exp,
                               scale=-1.0).then_inc(sSD, 1)

            def emit_r(nci):
                # r = 1/Z via exp(-ln(Z)); Z = po row 64
                act.wait_ge(sOA, MT * (nci + 1))
                if nci >= 1:
                    # WAR: PE's RB(nci-1) must have read r_sb
                    act.wait_ge(sUP, P[("rb", nci - 1, 1)] + 1)
                act.drain()
                act.activation(rln_sb[:], po[C:C + 1, :], AF.Ln)
                act.drain()
                act.activation(r_sb[:], rln_sb[:], AF.Exp,
                               scale=-1.0).then_inc(sR, 1)

            for g in range(NCH * MT):
                act.wait_ge(sL, g + 1)
                if g >= 2:
                    act.wait_ge(sOA, g - 1)
                act.activation(et[g % 2][:], lt[g % 2][:], AF.Exp).then_inc(sE, 1)
                if g == 20:
                    emit_etg()
                elif g == 22:
                    emit_rg()
                elif 24 <= g <= 31:
                    emit_esig(g - 24)
                if g % MT == MT - 1:
                    emit_r(g // MT)
                if g == 31:
                    emit_sig()


        @block.vector
        def _(dve):
            dve.wait_ge(sXIN, 16)
            dve.tensor_reduce(pooled_sb[:], xba[0:C, :], axis=AX.X,
                              op=ALU.max).then_inc(sPOOL, 1)
            dve.memset(xba[C:C + 1, :], 1.0).then_inc(sMS, 1)
            dve.memset(gaug_sb[C:C + 1, :], 1.0).then_inc(sMS, 1)
            dve.memset(xc[:], 0.0)
            # vT copies
            for t in range(MT):
                p = P[("vt", t)]
                dve.wait_ge(sUP, p + 1)
                dve.tensor_copy(vt_sb[:, 65 * t:65 * (t + 1)],
                                u[p % 2][0:128, 0:65]).then_inc(uc[p % 2], 1)
            # gca small ops
            p = P[("gat",)]
            dve.wait_ge(sUP, p + 1)
            dve.tensor_copy(gaug_sb[0:C, :],
                            u[p % 2][0:C, 0:4]).then_inc(uc[p % 2], 1)
            for tag, dst, rows, cols in [("vgt", vgt_sb, 4, 65),
                                         ("gq", qg_sb, 2, 4), ("gk", kg_sb, 2, 4)]:
                p = P[(tag,)]
                dve.wait_ge(sUP, p + 1)
                dve.tensor_copy(dst[:], u[p % 2][0:rows, 0:cols]).then_inc(uc[p % 2], 1)
                if tag == "gk":
                    dve.drain()
                    dve.memset(hB[0:1, 0:1], 0.0).then_inc(sGCAdve, 1)   # -> 1
            p = P[("outg",)]
            dve.wait_ge(sUP, p + 1)
            dve.tensor_copy(numt_sb[:], u[p % 2][0:4, 0:C])
            dve.tensor_copy(zg_sb[:], u[p % 2][0:4, C:C + 1]).then_inc(uc[p % 2], 1)
            dve.drain()
            dve.memset(hB[0:1, 0:1], 0.0).then_inc(sZG, 1)
            dve.wait_ge(sRG, 1)
            dve.tensor_scalar(gtmp_sb[:], numt_sb[:], rg_sb[:], gca_gamma,
                              op0=ALU.mult, op1=ALU.mult)
            dve.drain()
            dve.tensor_tensor(gpt_sb[:], gtmp_sb[:], gt_sb[:],
                              op=ALU.add).then_inc(sGCAdve, 1)   # -> 2
            # p1 = sig * x (sig computed on ACT)
            dve.wait_ge(sSD, 1)
            dve.tensor_tensor(p1_sb[:], sig_sb[:], xba[0:C, :], op=ALU.mult)
            # epilogues
            for nci in range(NCH):
                off = CW * nci
                # r computed on ACT (sR); S2 below waits on rb prods
                for h in range(2):
                    p = P[("rb", nci, h)]
                    dve.wait_ge(sUP, p + 1)
                    dve.tensor_tensor(s2_sb[:, 512 * h:512 * (h + 1)],
                                      sig_sb[:, off + 512 * h:off + 512 * (h + 1)],
                                      u[p % 2][0:C, :], op=ALU.mult).then_inc(uc[p % 2], 1)
                dve.drain()
                dve.tensor_tensor(g_sb[:], po[0:C, :], s2_sb[:],
                                  op=ALU.mult).then_inc(sEPo, 1)
                dve.drain()
                dve.tensor_tensor(xc[:, 1 + 16 * nci:1 + 16 * (nci + 1), 1:WB + 1],
                                  g_sb[:], p1_sb[:, off:off + CW],
                                  op=ALU.add).then_inc(sGATED, 1)
            # pack halo send strip (col 63, row 63, corner of gated tile)
            dve.drain()
            dve.tensor_copy(hsend_sb[:, 0:WB], xc[:, 1:HB + 1, WB])
            dve.tensor_copy(hsend_sb[:, WB:2 * WB], xc[:, HB, 1:WB + 1])
            dve.drain()
            dve.tensor_copy(hsend_sb[:, 2 * WB:2 * WB + 1],
                            xc[:, HB, WB:WB + 1]).then_inc(sHS, 1)
            # halo merge (mask-select the 3 neighbor strips)
            dve.wait_ge(sHG, 16)
            dve.wait_ge(sIN, 96)
            dve.tensor_tensor(hA[:], hg_sb[:, 0, :], hmask_sb[:, 0:129], op=ALU.mult)
            dve.tensor_tensor(hB[:], hg_sb[:, 1, :], hmask_sb[:, 129:258], op=ALU.mult)
            dve.tensor_tensor(hC[:], hg_sb[:, 2, :], hmask_sb[:, 258:387], op=ALU.mult)
            dve.tensor_tensor(hD[:], hg_sb[:, 3, :], hmask_sb[:, 387:516], op=ALU.mult)
            dve.drain()
            dve.tensor_tensor(hA[:], hA[:], hB[:], op=ALU.add)
            dve.tensor_tensor(hC[:], hC[:], hD[:], op=ALU.add)
            dve.drain()
            dve.tensor_tensor(hA[:], hA[:], hC[:], op=ALU.add)
            dve.drain()
            dve.tensor_copy(xc[:, 1:HB + 1, WB + 1], hA[:, 0:WB])
            dve.tensor_copy(xc[:, HB + 1, 1:WB + 1], hA[:, WB:2 * WB])
            dve.drain()
            dve.tensor_copy(xc[:, HB + 1, WB + 1:WB + 2],
                            hA[:, 2 * WB:2 * WB + 1]).then_inc(sHALO, 1)
            # conv epilogue
            for cch in range(8):
                p = P[("conv", cch)]
                dve.wait_ge(sUP, p + 1)
                dve.tensor_scalar(tA[cch % 2][:], u[p % 2][0:C, :], gamma, b2_sb[:],
                                  op0=ALU.mult, op1=ALU.add).then_inc(uc[p % 2], 1)
                dve.drain()
                dve.tensor_tensor(t2[cch % 2][:], tA[cch % 2][:],
                                  xba[0:C, 512 * cch:512 * (cch + 1)],
                                  op=ALU.add)
                dve.drain()
                if cch >= 2:
                    dve.wait_ge(sOD[cch % 2], 16 * (cch // 2))
                dve.tensor_scalar_max(osb[cch % 2][:],
                                      t2[cch % 2][:], 0.0).then_inc(sOUT, 1)

    return nc, ctx


_CACHE = {}


def kernel(**inputs):
    in_maps, sc = prep_inputs(inputs)
    key = (sc['nl_gamma'], sc['gca_gamma'], sc['gamma'])
    if key not in _CACHE:
        _CACHE[key] = build_nc(**sc)
    nc, _ctx = _CACHE[key]
    res = run_bass_kernel_spmd(nc, in_maps, core_ids=list(range(8)))
    outs = [res.results[i]["out"] for i in range(8)]
    return unshard(outs).astype(np.float32)


if __name__ == "__main__":
    nc, _ = build_nc(0.1, 0.1, 0.1)
    print("built ok;", len(nc.m.functions[0].allocations), "allocations")
